# revision 73
# baseline (speedup 1.0000x reference)
"""Causal single-head attention (B=4, T=4096, C=512, D=64) on 8 TRN2 NeuronCores.

Sharding: core c -> (batch b = c // 2, parity P = c % 2).  Each batch's 32
q-tiles (128 rows each) are striped by parity: core (b, P) owns global q-tiles
k = 2j + P, j = 0..15.  Slot j's causal kv extent is padded to 256*(j+1) keys
(uniform across parities) and the last 256 key columns get a parity-specific
additive mask fed as input data, so one SPMD program serves all 8 cores.

Precision plan (calibrated on HW probes): the x8-scaled score logits
(sigma ~64) make softmax weights exquisitely sensitive, so projections run
at fp32 quality via a 3-pass fp16 hi/lo split (x and W shipped as stacked
f16 hi|lo pairs; x_hi*W_hi + x_lo*W_hi + x_hi*W_lo at 1 cyc/col each vs
fp32's 4).  Q-hat/K-hat are stored float32r (tf32, one rounding); both score
passes run f32r at 1 cycle/col (pass-2 slot-paired so the moving dim is 256).
A and V are bf16 (rounding after exp / on values is harmless).

Per-core dataflow, slots processed in pairs (a=2i, b=2i+1):
  stream per i: project K/V chunk i (keys 512i..512i+512), project Q pair i,
  then flash pair i (its kv extent 512(i+1) is now resident):
  1. pass-1 per slot: S = Q_s K^T in f32r, 512-wide PSUM chunks; row-max per
     chunk on DVE (causal mask added to the last chunk first); the negated
     final max -> -m (bf16) -> [1,128] row via PE identity transpose -> QH
     partition 64 (f32r).  Emission is software-pipelined: pair i's pass-1
     interleaves with pair i-1's pass-2 so the in-order PE queue never
     stalls on the PE->DVE->PE max chain (keeps the p-state ramp at full
     clock; dummy warm-up matmuls cover the DMA-bound startup).
  2. pass-2 per pair: S^T - m via the 65-deep contraction [K^T;1]^T[Q^T;-m],
     moving = 256 q-cols (both slots) per key block; b-only tail blocks
     (last 512 keys) moving 128.  Transposed causal masks added on PSUM
     (DVE), one ACT exp (scale=8) per 1024-wide group writes A^T bf16.
  3. AV per slot: po[128, 65] += A^T_block^T V-hat_block (bf16); column 64
     accumulates l via V-hat's ones column; y = O / l on DVE.
"""

import numpy as np

B, T, C, D = 4, 4096, 512, 64
P128 = 128
NSLOT = 16          # q-tile slots per core
NPAIR = NSLOT // 2
TQ = NSLOT * P128   # 2048 q rows per core
NEG = -1.0e30

_CACHED = {}


def _build(RATIO=3.0):
    import concourse.mybir as mybir
    from concourse import bacc
    from concourse.tile import TileContext
    from concourse.masks import make_identity

    f32 = mybir.dt.float32
    f32r = mybir.dt.float32r
    f16 = mybir.dt.float16
    bf16 = mybir.dt.bfloat16
    AX = mybir.AxisListType.X
    ALU = mybir.AluOpType
    ACTF = mybir.ActivationFunctionType

    nc = bacc.Bacc("TRN2", target_bir_lowering=False, debug=False,
                   enable_asserts=False, num_devices=8)

    xT = nc.dram_tensor("xT", [2 * C, T], f16, kind="ExternalInput").ap()
    xTq = nc.dram_tensor("xTq", [2 * C, TQ], f16, kind="ExternalInput").ap()
    wqh = nc.dram_tensor("wqh", [2 * C, D], f16, kind="ExternalInput").ap()
    wkv2 = nc.dram_tensor("wkv2", [2 * C, 2 * D], f16, kind="ExternalInput").ap()
    bq = nc.dram_tensor("bq", [D, 1], f32, kind="ExternalInput").ap()
    bk = nc.dram_tensor("bk", [D, 1], f32, kind="ExternalInput").ap()
    bv = nc.dram_tensor("bv", [D, 1], f32, kind="ExternalInput").ap()
    maskp = nc.dram_tensor("maskp", [P128, 512], f32, kind="ExternalInput").ap()
    maskpT = nc.dram_tensor("maskpT", [P128, 256], f32, kind="ExternalInput").ap()
    onesr = nc.dram_tensor("onesr", [1, T], f32r, kind="ExternalInput").ap()
    y = nc.dram_tensor("y", [TQ, D], f32, kind="ExternalOutput").ap()
    DV = D + 1  # V tiles carry a ones column

    with TileContext(nc) as tc:
        with (
            tc.tile_pool(name="singles", bufs=1) as singles,
            tc.tile_pool(name="xin", bufs=3) as xin,
            tc.tile_pool(name="qin", bufs=2) as qin,
            tc.tile_pool(name="atp", bufs=2) as atp,
            tc.tile_pool(name="vwork", bufs=3) as vwork,
            tc.tile_pool(name="small", bufs=6) as small,
            tc.tile_pool(name="ps_s", bufs=3, space="PSUM") as ps_s,
            tc.tile_pool(name="ps_e", bufs=2, space="PSUM") as ps_e,
            tc.tile_pool(name="ps_o", bufs=2, space="PSUM") as ps_o,
            tc.tile_pool(name="ps_m", bufs=1, space="PSUM") as ps_m,
        ):
            # ---- resident constants ----
            wqs = singles.tile([P128, 8, D], f16, tag="wqs")
            wkv = singles.tile([P128, 8, 2 * D], f16, tag="wkv")
            nc.sync.dma_start(out=wkv, in_=wkv2.rearrange("(c p) d -> p c d", p=P128))
            nc.sync.dma_start(out=wqs, in_=wqh.rearrange("(c p) d -> p c d", p=P128))
            bqs = singles.tile([D, 1], f32, tag="bqs")
            bks = singles.tile([D, 1], f32, tag="bks")
            bvs = singles.tile([D, 1], f32, tag="bvs")
            nc.sync.dma_start(out=bqs, in_=bq)
            nc.sync.dma_start(out=bks, in_=bk)
            nc.sync.dma_start(out=bvs, in_=bv)
            msk = singles.tile([P128, 512], f32, tag="msk")
            mskT = singles.tile([P128, 256], f32, tag="mskT")
            identb = singles.tile([P128, P128], bf16, tag="identb")
            make_identity(nc, identb)
            identh = singles.tile([P128, P128], f16, tag="identh")
            make_identity(nc, identh)

            # K-hat [K^T; ones] / Q-hat [Q^T; -m] in f32r (tf32-rounded once)
            KH = singles.tile([D + 1, T], f32r, tag="KH")
            QH = singles.tile([D + 1, TQ], f32r, tag="QH")

            Vsb = singles.tile([P128, (T // P128) * DV], bf16, tag="Vsb")
            nc.gpsimd.memset(Vsb, 1.0)   # ones column survives V writes
            scr = singles.tile([1, 1], f32, tag="scr")
            nc.gpsimd.memset(scr, 0.0)
            nc.scalar.activation(scr, scr, ACTF.Exp, bias=0.0, scale=1.0)
            # keep PE continuously busy through the DMA-bound startup so the
            # p-state ramp reaches full clock before real work arrives
            warm = ps_m.tile([P128, 512], f32, tag="pm")
            for _ in range(64):
                nc.tensor.matmul(warm[:, :P128], identb, identb,
                                 start=True, stop=True)

            xts, xqs = {}, {}

            def load_kv(t8):
                xts[t8] = xin.tile([P128, 8, 512], f16, tag="xt", name=f"xt{t8}")
                nc.sync.dma_start(
                    out=xts[t8],
                    in_=xT[:, t8 * 512:(t8 + 1) * 512].rearrange(
                        "(c p) n -> p c n", p=P128))

            def load_q(i):
                # pair i's q columns: [256*i, 256*i+256)
                xqs[i] = qin.tile([P128, 8, 256], f16, tag="xq", name=f"xq{i}")
                nc.sync.dma_start(
                    out=xqs[i],
                    in_=xTq[:, i * 256:(i + 1) * 256].rearrange(
                        "(c p) n -> p c n", p=P128))

            def proj_kv(t8):
                xt = xts.pop(t8)
                kvps = ps_s.tile([P128, 512], f32, tag="ps")
                pairs = ([(c, c) for c in range(4)]
                         + [(c, c + 4) for c in range(4)]
                         + [(c + 4, c) for c in range(4)])
                for n, (wc, xc) in enumerate(pairs):
                    nc.tensor.matmul(kvps, wkv[:, wc, :], xt[:, xc, :],
                                     start=(n == 0), stop=(n == len(pairs) - 1))
                nc.scalar.activation(KH[:D, t8 * 512:(t8 + 1) * 512],
                                     kvps[:D, :], ACTF.Identity,
                                     bias=bks, scale=1.0)
                vtmp = vwork.tile([D, 512], bf16, tag="vtmp")
                nc.scalar.activation(vtmp, kvps[D:, :], ACTF.Identity,
                                     bias=bvs, scale=1.0)
                # transpose V^T [64, 128] blocks -> bf16 V-hat [128, 65] tiles
                for i in range(4):
                    t = t8 * 4 + i
                    pt = ps_m.tile([P128, 512], bf16, tag="pm")
                    nc.tensor.transpose(pt[:, :D],
                                        vtmp[:, i * P128:(i + 1) * P128],
                                        identb[:D, :D])
                    nc.vector.tensor_copy(Vsb[:, t * DV:t * DV + D], pt[:, :D])

            def proj_q(i):
                # pair i's Q: 256 columns
                xt = xqs.pop(i)
                qps = ps_s.tile([P128, 512], f32, tag="ps")
                pairs = ([(c, c) for c in range(4)]
                         + [(c, c + 4) for c in range(4)]
                         + [(c + 4, c) for c in range(4)])
                for n, (wc, xc) in enumerate(pairs):
                    nc.tensor.matmul(qps[:D, :256], wqs[:, wc, :], xt[:, xc, :],
                                     start=(n == 0), stop=(n == len(pairs) - 1))
                nc.scalar.activation(QH[:D, i * 256:(i + 1) * 256],
                                     qps[:D, :256], ACTF.Identity,
                                     bias=bqs, scale=1.0)

            def gen_pass1_pair(i):
                """Row max per slot of pair i; writes -m into QH.  Yields
                between emission units so pass-2 of the previous pair can be
                interleaved into the in-order PE queue (keeps PE busy while
                DVE drains pass-1 PSUM chunks; preserves the p-state ramp)."""
                for s in (2 * i, 2 * i + 1):
                    ncols = 256 * (s + 1)
                    chunks = [(off, min(512, ncols - off))
                              for off in range(0, ncols, 512)]
                    mgp = small.tile([P128, 8], f32, tag="mgp")
                    ci = 0
                    qsl = QH[:D, s * P128:(s + 1) * P128]
                    for (off, sw) in chunks[:-1]:
                        ps = ps_s.tile([P128, 512], f32, tag="ps")
                        nc.tensor.matmul(ps, qsl, KH[:D, off:off + sw],
                                         start=True, stop=True)
                        nc.vector.reduce_max(mgp[:, ci:ci + 1], ps, axis=AX)
                        ci += 1
                        yield
                    off, sw = chunks[-1]
                    ps = ps_s.tile([P128, 512], f32, tag="ps")
                    nc.tensor.matmul(ps[:, :sw], qsl, KH[:D, off:off + sw],
                                     start=True, stop=True)
                    nc.vector.tensor_add(ps[:, :sw], ps[:, :sw],
                                         msk[:, 512 - sw:])
                    nc.vector.reduce_max(mgp[:, ci:ci + 1], ps[:, :sw],
                                         axis=AX)
                    ci += 1
                    yield
                    mrunh = small.tile([P128, 1], bf16, tag="mrunh")
                    nc.vector.reduce_max(mrunh, mgp[:, :ci], axis=AX,
                                         negate=True)
                    # -m -> [1,128] row via PE transpose -> QH partition 64
                    pm = ps_m.tile([P128, 512], bf16, tag="pm")
                    nc.tensor.transpose(pm[:1, :P128], mrunh, identb)
                    nc.vector.tensor_copy(
                        QH[D:D + 1, s * P128:(s + 1) * P128], pm[:1, :P128])
                    yield

            def gen_pass2_pair(i):
                a, b = 2 * i, 2 * i + 1
                nfull = 4 * i + 2          # key blocks where both slots attend
                ecols = nfull * 256 + 256  # pass-2 staging cols incl. b tails
                while pend_finals:
                    pend_finals.pop(0)()
                AT = atp.tile([P128, 8192], bf16, tag="AT")
                qpr = QH[:, a * P128:a * P128 + 256]
                po = ps_o.tile([P128, 512], f32, tag="po", name="po_a")
                pob = ps_o.tile([P128, 512], f32, tag="po", name="po_b")
                av_a = av_b = 0

                last_goff = max(0, (ecols - 1) // 512 * 512 - 512)
                goff = 0
                while goff < ecols:
                    gw = min(512, ecols - goff)
                    ps2 = ps_e.tile([P128, 512], f32, tag="ps2")
                    seg = goff
                    while seg < goff + gw:
                        kb = seg // 256
                        if kb < nfull:
                            nc.tensor.matmul(
                                ps2[:, seg - goff:seg - goff + 256],
                                KH[:, kb * P128:(kb + 1) * P128], qpr,
                                start=True, stop=True)
                        else:
                            # two b-only tail blocks, 128 cols each
                            for tix in range(2):
                                kb2 = nfull + tix
                                so = seg - goff + tix * P128
                                nc.tensor.matmul(
                                    ps2[:, so:so + P128],
                                    KH[:, kb2 * P128:(kb2 + 1) * P128],
                                    QH[:, b * P128:(b + 1) * P128],
                                    start=True, stop=True)
                        seg += 256
                        yield
                    # causal masks on PSUM before exp
                    lo, hi = goff, goff + gw
                    m0 = (4 * i) * 256          # a-half of block 4i
                    if lo <= m0 < hi:
                        nc.vector.tensor_add(
                            ps2[:, m0 - goff:m0 - goff + P128],
                            ps2[:, m0 - goff:m0 - goff + P128],
                            mskT[:, 0:P128])
                    m1 = (4 * i + 1) * 256      # a-half of block 4i+1
                    if lo <= m1 < hi:
                        nc.vector.tensor_add(
                            ps2[:, m1 - goff:m1 - goff + P128],
                            ps2[:, m1 - goff:m1 - goff + P128],
                            mskT[:, P128:256])
                    mt = nfull * 256            # b tails
                    if lo <= mt < hi:
                        nc.vector.tensor_add(
                            ps2[:, mt - goff:mt - goff + 256],
                            ps2[:, mt - goff:mt - goff + 256],
                            mskT)
                    nc.scalar.activation(AT[:, goff:goff + gw], ps2[:, :gw],
                                         ACTF.Exp, bias=0.0, scale=8.0)
                    yield
                    # AV incrementally for both slots (separate PSUM banks);
                    # the FINAL group's AVs are deferred into the next pair's
                    # stream so the end-of-pair exp latency is covered
                    if goff < last_goff:
                        done = goff + gw
                        while (av_a + 1) * 256 <= done and av_a < nfull:
                            blk = av_a
                            nc.tensor.matmul(
                                po[:, :DV], AT[:, blk * 256:blk * 256 + P128],
                                Vsb[:, blk * DV:(blk + 1) * DV],
                                start=(blk == 0), stop=(blk == nfull - 1))
                            av_a += 1
                        while av_b < nfull and (av_b + 1) * 256 <= done:
                            blk = av_b
                            st = AT[:, blk * 256 + P128:(blk + 1) * 256]
                            nc.tensor.matmul(
                                pob[:, :DV], st,
                                Vsb[:, blk * DV:(blk + 1) * DV],
                                start=(blk == 0), stop=(blk == nfull + 1))
                            av_b += 1
                    yield
                    goff += gw

                def tail(a0=av_a, b0=av_b):
                    blk = a0
                    while blk < nfull:
                        nc.tensor.matmul(
                            po[:, :DV], AT[:, blk * 256:blk * 256 + P128],
                            Vsb[:, blk * DV:(blk + 1) * DV],
                            start=(blk == 0), stop=(blk == nfull - 1))
                        blk += 1
                    blk = b0
                    while blk < nfull + 2:
                        if blk < nfull:
                            st = AT[:, blk * 256 + P128:(blk + 1) * 256]
                        else:
                            st = AT[:, nfull * 256 + (blk - nfull) * P128:
                                    nfull * 256 + (blk - nfull + 1) * P128]
                        nc.tensor.matmul(
                            pob[:, :DV], st,
                            Vsb[:, blk * DV:(blk + 1) * DV],
                            start=(blk == 0), stop=(blk == nfull + 1))
                        blk += 1
                    for s, pot in ((a, po), (b, pob)):
                        rl = small.tile([P128, 1], f32, tag="rl")
                        nc.vector.reciprocal(rl, pot[:, D:D + 1])
                        yt = small.tile([P128, D], f32, tag="yt")
                        nc.vector.tensor_scalar_mul(yt, pot[:, :D], rl)
                        nc.sync.dma_start(out=y[s * P128:(s + 1) * P128, :],
                                          in_=yt)
                pend_finals.append(tail)
                yield

            def drain(gen):
                for _ in gen:
                    pass

            def interleave(g1, g2, ratio=3.0):
                """Pump g2 `ratio` units per g1 unit (fractional pacing) so
                the pass-2 filler material lasts through all of pass-1."""
                done1 = done2 = False
                credit = 0.0
                while not (done1 and done2):
                    if not done1:
                        done1 = next(g1, _SENTINEL) is _SENTINEL
                    credit += ratio
                    while credit >= 1.0 and not done2:
                        done2 = next(g2, _SENTINEL) is _SENTINEL
                        credit -= 1.0

            _SENTINEL = object()

            # ---- software-pipelined schedule ----
            load_kv(0)
            load_q(0)
            load_kv(1)
            nc.sync.dma_start(out=msk, in_=maskp)
            load_q(1)
            nc.sync.dma_start(out=mskT, in_=maskpT)
            nc.sync.dma_start(out=KH[D:D + 1, :], in_=onesr)
            prev = None
            pend_finals = []
            for i in range(NPAIR):
                if i + 2 < NPAIR:
                    load_kv(i + 2)
                if i + 2 < NPAIR:
                    load_q(i + 2)
                proj_kv(i)
                proj_q(i)
                g1 = gen_pass1_pair(i)
                if prev is None:
                    drain(g1)
                else:
                    interleave(g1, prev, ratio=RATIO)
                prev = gen_pass2_pair(i)
            drain(prev)
            while pend_finals:
                pend_finals.pop(0)()

    nc.compile()
    return nc


def _get_nc():
    if "nc" not in _CACHED:
        _CACHED["nc"] = _build()
    return _CACHED["nc"]


def _prep_in_maps(x, Wq, bq, Wk, bk, Wv, bv):
    x = np.asarray(x, dtype=np.float32)
    Wq = np.asarray(Wq, dtype=np.float32)
    Wk = np.asarray(Wk, dtype=np.float32)
    Wv = np.asarray(Wv, dtype=np.float32)
    bq_ = np.asarray(bq, dtype=np.float32).reshape(D, 1)
    bk_ = np.asarray(bk, dtype=np.float32).reshape(D, 1)
    bv_ = np.asarray(bv, dtype=np.float32).reshape(D, 1)

    tri = np.triu(np.ones((P128, P128), np.float32), k=1) * NEG
    masks = []
    for P in range(2):
        mp = np.zeros((P128, 512), np.float32)
        if P == 0:
            mp[:, 256:384] = tri
            mp[:, 384:512] = NEG
        else:
            mp[:, 384:512] = tri
        masks.append(mp)

    masksT = []
    for P in range(2):
        mt = np.zeros((P128, 256), np.float32)
        mt[:, 0:128] = masks[P][:, 256:384].T
        mt[:, 128:256] = masks[P][:, 384:512].T
        masksT.append(mt)

    def hilo(a):
        """Stack [M, N] fp32 -> [2M, N] fp16 (hi rows, then residual rows)."""
        hi = a.astype(np.float16)
        lo = (a - hi.astype(np.float32)).astype(np.float16)
        return np.concatenate([hi, lo], axis=0)

    ones_row = np.ones((1, T), np.float32)
    wq2 = hilo(Wq)
    wkv2 = hilo(np.concatenate([Wk, Wv], axis=1))
    xTs = {}
    xTqs = {}
    in_maps = []
    for c in range(8):
        b, P = c // 2, c % 2
        if b not in xTs:
            xTs[b] = hilo(np.ascontiguousarray(x[b].T))
        if (b, P) not in xTqs:
            rows = ((np.arange(NSLOT) * 2 + P)[:, None] * P128
                    + np.arange(P128)[None, :]).reshape(-1)
            xTqs[(b, P)] = hilo(np.ascontiguousarray(x[b][rows].T))
        in_maps.append({
            "xT": xTs[b],
            "xTq": xTqs[(b, P)],
            "wqh": wq2, "wkv2": wkv2,
            "bq": bq_, "bk": bk_, "bv": bv_,
            "maskp": masks[P], "maskpT": masksT[P],
            "onesr": ones_row,
        })
    return in_maps


def _unshard(res):
    out = np.empty((B, T, D), np.float32)
    for c in range(8):
        b, P = c // 2, c % 2
        yl = res.results[c]["y"]
        for j in range(NSLOT):
            k = 2 * j + P
            out[b, k * P128:(k + 1) * P128] = yl[j * P128:(j + 1) * P128]
    return out


def kernel(x, Wq, bq, Wk, bk, Wv, bv):
    from concourse.bass_utils import run_bass_kernel_spmd

    in_maps = _prep_in_maps(x, Wq, bq, Wk, bk, Wv, bv)
    res = run_bass_kernel_spmd(_get_nc(), in_maps, core_ids=list(range(8)))
    _CACHED["last_results"] = res
    return _unshard(res)


if __name__ == "__main__":
    rng = np.random.default_rng(0)
    x = rng.standard_normal((B, T, C), dtype=np.float32)
    s = 1.0 / np.sqrt(C)
    Wq = rng.standard_normal((C, D), dtype=np.float32) * s
    Wk = rng.standard_normal((C, D), dtype=np.float32) * s
    Wv = rng.standard_normal((C, D), dtype=np.float32) * s
    z = np.zeros(D, np.float32)
    print(kernel(x, Wq, z, Wk, z, Wv, z).shape)
